# revision 43
# baseline (speedup 1.0000x reference)
"""Multi-head self-attention (1x1-conv projections, N=4096 spatial tokens,
C=256 channels, Cq=32) on 8 TRN2 NeuronCores, data-parallel over batch.

Per core (one batch element, x as [C, N]):
  q = wq @ x + bq          [Cq, N]
  k = wk @ x + bk          [Cq, N]
  v = wv @ x + bv          [C, N]   (bv folded into the epilogue)
  S = q^T k; P = softmax(S, axis=-1)
  out = gamma * (v @ P^T) + gamma*bv + x

Layout: S^T tiles (keys j on partitions, queries i free) so exp(S^T) feeds
the PV matmul as the stationary operand with rhs = [v^T | ones]; the ones
column accumulates the softmax denominator for free.  exp skips
max-subtraction (S within +-45, safe in fp32/bf16).

Energy/softmax pipeline: j-tile PAIRS over 512-query super-blocks, each
pair one 2-bank PSUM tile from a THREE-deep rotation (6 banks).  Depth 3
keeps ACT's exp stream saturated (the pair's write-after-read wait on an
exp two steps back resolves early), pairs alternate PE row-group halves so
consecutive pairs run concurrently into distinct banks (concurrent
same-bank drains wedge the PE), and one exp covers each pair's 1024
elements.  PV consumes 256-query i-blocks (two [128,257] accumulators,
2 banks) whose epilogue transposes rotate through the same pool.
Projections use a scoped prologue pool whose banks are reclaimed by the
attention pools; a ~4us warm-up matmul burst opens the HAM clock gate
before real work lands; weights/consts issue on scalar/gpsimd DMA queues
so the x loads own the sync queue; x is loaded ONCE (fp32r view for the
projections, residual formed into a separate tile).
"""

import numpy as np

import concourse.bass as bass
import concourse.mybir as mybir
import concourse.tile as tile
from concourse.bass_utils import run_bass_kernel_spmd
from concourse.masks import make_identity
from concourse.tile import ScopedClock

F32 = mybir.dt.float32
F32R = mybir.dt.float32r
BF16 = mybir.dt.bfloat16

B, C, CQ = 8, 256, 32
H = W = 64
N = H * W            # 4096 tokens
NCORES = 8
CT = C // 128        # 2 channel tiles
IB = 256             # queries per PV i-block
N_IB = N // IB       # 16
JT = N // 128        # 32 key tiles
SB = 512             # queries per energy super-block (2 PV i-blocks)
NSB = N // SB        # 8
NPAIR = JT // 2      # 16 j-tile pairs per super-block
CH = 512             # x columns per load/projection chunk
N_CH = N // CH       # 8


class PatchedTileContext(tile.TileContext):
    """This walrus build supports only ONE sync-wait command per
    instruction. Peel extra waits into standalone single-wait NOPs on the
    same engine queue, emitted immediately before the instruction (a serial
    conjunction of waits - semantically identical). Same treatment for the
    kernel-tail drain, whose global-clock waits otherwise all land on one
    Drain instruction."""

    MAX_WAITS_PER_INST = 1

    def _add_instruction(self, inst):
        si = inst.sync_info
        waits = list(si.on_wait) if si is not None and si.on_wait else []
        if len(waits) > self.MAX_WAITS_PER_INST and inst.engine is not None:
            keep = waits[-self.MAX_WAITS_PER_INST:]
            peel = waits[: -self.MAX_WAITS_PER_INST]
            for w in peel:
                nop = mybir.InstNoOp(
                    name=self.nc.get_next_instruction_name(),
                    ins=[],
                    outs=[],
                    sync_info=mybir.SyncInfo(on_wait=[w], on_update=[]),
                )
                nop.engine = inst.engine
                super()._add_instruction(nop)
            inst.sync_info = mybir.SyncInfo(
                on_wait=keep,
                on_update=list(si.on_update) if si.on_update else [],
            )
        super()._add_instruction(inst)

    def _drain_and_barrier(self, tick_clock, wait_clock):
        nc = self.nc
        carrier = nc.sync.nop()
        wait_clock.add_sem_waits(
            carrier.ins, ScopedClock({None: tick_clock.global_clock})
        )
        si = carrier.ins.sync_info
        waits = list(si.on_wait) if si is not None and si.on_wait else []
        carrier.ins.sync_info = None
        for w in waits:
            h = bass.SemaphoreHandle(name=w.ant_name or f"sem{w.id}", num=w.id)
            if w.wait_mode == "sem-ge-imm":
                nc.sync.wait_ge(h, w.wait_value)
            else:
                op = {
                    "sem-eq-imm": "eq",
                    "sem-le-imm": "le",
                    "sem-lt-imm": "lt",
                    "sem-gt-imm": "gt",
                }[w.wait_mode]
                nc.sync.wait_op(h, w.wait_value, op)
        nc.sync.drain()
        nc.all_engine_barrier()
        assert self.sems is not None
        popped = nc._tile_sem_poison_stack.pop()
        assert popped is self._sem_poison
        nc.clear_and_free_semaphores(list(self.sems.allocated().values()))
        nc.all_engine_barrier()


def _attention_body(nc, tc, ctx):
    x_e = nc.dram_tensor("x", [C, N], F32, kind="ExternalInput")
    wqt4_e = nc.dram_tensor("wqt4", [C, 128], F32, kind="ExternalInput")
    wkt4_e = nc.dram_tensor("wkt4", [C, 128], F32, kind="ExternalInput")
    wvt_e = nc.dram_tensor("wvt", [C, C], F32, kind="ExternalInput")
    bq4_e = nc.dram_tensor("bq4", [128, 1], F32, kind="ExternalInput")
    bk4_e = nc.dram_tensor("bk4", [128, 1], F32, kind="ExternalInput")
    bv_e = nc.dram_tensor("bv2", [128, CT], F32, kind="ExternalInput")
    gamma_e = nc.dram_tensor("gamma128", [128, 1], F32, kind="ExternalInput")
    out_e = nc.dram_tensor("out", [C, N], F32, kind="ExternalOutput")

    x_v = x_e.rearrange("(t p) n -> p t n", p=128)      # [128, CT, N]
    out_v = out_e.rearrange("(t p) n -> p t n", p=128)  # [128, CT, N]
    wqt_v = wqt4_e.rearrange("(t p) m -> p t m", p=128)
    wkt_v = wkt4_e.rearrange("(t p) m -> p t m", p=128)
    wvt_v = wvt_e.rearrange("(t p) m -> p t m", p=128)

    const = ctx.enter_context(tc.tile_pool(name="const", bufs=1))
    sb = ctx.enter_context(tc.tile_pool(name="sb", bufs=1))
    Ep = ctx.enter_context(tc.tile_pool(name="Ep", bufs=2))
    eps = ctx.enter_context(tc.tile_pool(name="eps", bufs=4))
    outp = ctx.enter_context(tc.tile_pool(name="outp", bufs=2))

    # ---- constants / weights (weights on the scalar queue, small consts
    # on gpsimd: each dma_start costs ~0.65us of issue time on its queue;
    # seven of them ahead of the x loads on sync delay the first matmul)
    bq4 = const.tile([128, 1], F32)
    bk4 = const.tile([128, 1], F32)
    bv2 = const.tile([128, CT], F32)
    gamma = const.tile([128, 1], F32)
    wq_f = const.tile([128, CT, 128], F32R)
    wk_f = const.tile([128, CT, 128], F32R)
    wv_f = const.tile([128, CT, C], F32R)
    nc.scalar.dma_start(out=wq_f, in_=wqt_v.bitcast(F32R))
    nc.scalar.dma_start(out=wk_f, in_=wkt_v.bitcast(F32R))
    nc.scalar.dma_start(out=wv_f, in_=wvt_v.bitcast(F32R))
    nc.gpsimd.dma_start(out=bq4, in_=bq4_e[:, :])
    nc.gpsimd.dma_start(out=bk4, in_=bk4_e[:, :])
    nc.gpsimd.dma_start(out=bv2, in_=bv_e[:, :])
    nc.gpsimd.dma_start(out=gamma, in_=gamma_e[:, :])

    ident = const.tile([128, 128], BF16)
    make_identity(nc, ident)

    gbv = const.tile([128, CT], F32)
    nc.vector.tensor_scalar(
        out=gbv, in0=bv2, scalar1=gamma, scalar2=None, op0=mybir.AluOpType.mult
    )

    x_sb = sb.tile([128, CT, N], F32R)  # x, loaded once (fp32r view)
    xb_sb = sb.tile([128, CT, N], F32)  # x + gamma*bv (residual)
    qT = sb.tile([128, N], F32R)        # q^T replicated on 4 partition groups
    kT = sb.tile([128, N], F32R)
    v1T = sb.tile([128, JT, C + 1], BF16)  # [j-part, j-tile, c | ones]
    nc.vector.memset(v1T[:, :, C : C + 1], 1.0)

    # ACT exp-table preload: dummy exp (cell overwritten by the residual
    # adds, which gives it a reader) pulls the ~2.7us table DMA forward
    zt = const.tile([128, 1], F32)
    nc.vector.memset(zt, 0.0)
    nc.scalar.activation(
        out=xb_sb[:, 0, 0:1], in_=zt, func=mybir.ActivationFunctionType.Exp
    )

    # ---- prologue: projections (psA banks are reclaimed afterwards) ----
    with tc.tile_pool(name="psA", bufs=4, space="PSUM") as psA:
        # HAM warm-up: ~4us of dependency-free back-to-back matmuls so the
        # PE clock gate opens (1.2 -> 2.4 GHz) before the real work lands
        wu = const.tile([128, 1024], BF16)
        nc.vector.memset(wu, 0.0)
        pwu = psA.tile([128, 512], F32, tag="pj", name="pwu")
        for _ in range(9):
            nc.tensor.matmul(
                pwu, wu[:, 0:128], wu[:, 0:512], start=True, stop=True
            )
        for ch in range(N_CH):
            sl = bass.ts(ch, CH)
            nc.sync.dma_start(
                out=x_sb[:, :, sl], in_=x_v[:, :, sl].bitcast(F32R)
            )
            pq = psA.tile([128, CH], F32, tag="pj", name=f"pq_{ch}")
            nc.tensor.matmul(pq, wq_f[:, 0, :], x_sb[:, 0, sl], start=True, stop=False)
            nc.tensor.matmul(pq, wq_f[:, 1, :], x_sb[:, 1, sl], start=False, stop=True)
            nc.vector.tensor_scalar(
                out=qT[:, sl], in0=pq, scalar1=bq4, scalar2=None,
                op0=mybir.AluOpType.add,
            )
            pk = psA.tile([128, CH], F32, tag="pj", name=f"pk_{ch}")
            nc.tensor.matmul(pk, wk_f[:, 0, :], x_sb[:, 0, sl], start=True, stop=False)
            nc.tensor.matmul(pk, wk_f[:, 1, :], x_sb[:, 1, sl], start=False, stop=True)
            nc.vector.tensor_scalar(
                out=kT[:, sl], in0=pk, scalar1=bk4, scalar2=None,
                op0=mybir.AluOpType.add,
            )
            for nt in range(4 * ch, 4 * ch + 4):
                pv = psA.tile([128, C], F32, tag="pj", name=f"pv_{nt}")
                nc.tensor.matmul(
                    pv, x_sb[:, 0, bass.ts(nt, 128)], wv_f[:, 0, :],
                    start=True, stop=False,
                )
                nc.tensor.matmul(
                    pv, x_sb[:, 1, bass.ts(nt, 128)], wv_f[:, 1, :],
                    start=False, stop=True,
                )
                nc.vector.tensor_copy(out=v1T[:, nt, 0:C], in_=pv)
            # residual with bv folded in, after this chunk's last x read
            for t in range(CT):
                nc.vector.tensor_scalar(
                    out=xb_sb[:, t, sl],
                    in0=x_sb[:, t, sl].bitcast(F32),
                    scalar1=gbv[:, t : t + 1], scalar2=None,
                    op0=mybir.AluOpType.add,
                )

    # ---- attention ----
    E_of = {}

    def E_tile(sbk):
        if sbk not in E_of:
            E_of[sbk] = Ep.tile([128, JT, SB], BF16, tag="E", name=f"E_{sbk}")
        return E_of[sbk]

    po_of = {}

    def po_tiles(ib):
        if ib not in po_of:
            po_of[ib] = [
                psO.tile([128, C + 1], F32, tag="acc", name=f"po_{ib}_{i}")
                for i in range(IB // 128)
            ]
        return po_of[ib]

    pair_seq = [(s, p) for s in range(NSB) for p in range(NPAIR)]
    pair_idx = [0]

    def emit_pair():
        # one j-tile pair (jt 2p, 2p+1) x 512 queries of super-block s,
        # into a 2-bank tile of the 3-deep psS rotation.  The two matmuls
        # drain into the tile's two DISTINCT banks; consecutive pairs use
        # opposite PE row-group halves so they overlap in the array.
        s, p = pair_seq[pair_idx[0]]
        pair_idx[0] += 1
        isl = bass.ds(s * SB, SB)
        S = psS.tile([128, 2, SB], F32, tag="S", name=f"S_{s}_{p}")
        for g in range(2):
            jt = 2 * p + g
            gp = bass.ds(64 * (p % 2) + 32 * g, 32)
            nc.tensor.matmul(
                S[:, g, :],
                kT[gp, bass.ts(jt, 128)],
                qT[gp, isl],
                start=True, stop=True,
                tile_position=(64 * (p % 2) + 32 * g, 0),
            )
        nc.scalar.activation(
            out=E_tile(s)[:, 2 * p : 2 * p + 2, :],
            in_=S,
            func=mybir.ActivationFunctionType.Exp,
        )

    def drip_pairs(k, ib):
        # feed the energy/exp pipeline, never past super-block ib//2 + 1
        # (the E pool is double-buffered)
        while (
            k > 0
            and pair_idx[0] < len(pair_seq)
            and pair_seq[pair_idx[0]][0] <= ib // 2 + 1
        ):
            emit_pair()
            k -= 1

    def emit_pv(ib, slot):
        po = po_tiles(ib)
        E = E_tile(ib // 2)
        ioff = (ib % 2) * IB
        for jt in range(4 * slot, 4 * slot + 4):
            for i_s in range(IB // 128):
                nc.tensor.matmul(
                    po[i_s],
                    E[:, jt, bass.ds(ioff + i_s * 128, 128)],
                    v1T[:, jt, :],
                    start=(jt == 0), stop=(jt == JT - 1),
                )

    def epilogue(ib):
        po = po_tiles(ib)
        ot = outp.tile([128, CT, IB], F32, tag="ot")
        for i_s in range(IB // 128):
            rd = eps.tile([128, 1], F32, tag="rd")
            nc.vector.reciprocal(out=rd, in_=po[i_s][:, C : C + 1])
            nc.vector.tensor_mul(out=rd, in0=rd, in1=gamma)
            pvn = eps.tile([128, C], BF16, tag="pvn")
            nc.vector.tensor_scalar(
                out=pvn, in0=po[i_s][:, 0:C], scalar1=rd, scalar2=None,
                op0=mybir.AluOpType.mult,
            )
            pt = psO.tile([128, C], BF16, tag="acc", name=f"pt_{ib}_{i_s}")
            nc.tensor.transpose(pt[:, 0:128], pvn[:, 0:128], ident)
            nc.tensor.transpose(pt[:, 128:256], pvn[:, 128:256], ident)
            for t in range(CT):
                nc.vector.tensor_add(
                    out=ot[:, t, bass.ts(i_s, 128)],
                    in0=pt[:, bass.ts(t, 128)],
                    in1=xb_sb[:, t, bass.ds(ib * IB + i_s * 128, 128)],
                )
        for t in range(CT):
            nc.sync.dma_start(out=out_v[:, t, bass.ts(ib, IB)], in_=ot[:, t, :])

    with (
        tc.tile_pool(name="psS", bufs=3, space="PSUM") as psS,
        tc.tile_pool(name="psO", bufs=2, space="PSUM") as psO,
    ):
        drip_pairs(3, 0)   # fill the 3-deep rotation
        for ib in range(N_IB):
            for slot in range(8):
                emit_pv(ib, slot)
                drip_pairs(2, ib)
            epilogue(ib)
        # drain any pairs the cap held back (none expected)
        drip_pairs(len(pair_seq), N_IB)


_CACHE = {}


def _build():
    if "nc" not in _CACHE:
        nc = bass.Bass()
        from contextlib import ExitStack
        with PatchedTileContext(nc) as tc, ExitStack() as ctx:
            _attention_body(nc, tc, ctx)
        _CACHE["nc"] = nc
    return _CACHE["nc"]


def _prep_in_maps(x, wq, bq, wk, bk, wv, bv, gamma):
    asc = np.ascontiguousarray
    wqt4 = asc(np.tile(wq, (4, 1)).T.astype(np.float32))    # [C, 128]
    wkt4 = asc(np.tile(wk, (4, 1)).T.astype(np.float32))    # [C, 128]
    wvt = asc(wv.T.astype(np.float32))                      # [C, C]
    bq4 = asc(np.tile(bq, 4)[:, None].astype(np.float32))   # [128, 1]
    bk4 = asc(np.tile(bk, 4)[:, None].astype(np.float32))
    bv2 = asc(bv.reshape(CT, 128).T.astype(np.float32))     # [128, CT]
    g128 = np.full((128, 1), np.float32(gamma[0]), dtype=np.float32)
    maps = []
    for b in range(B):
        maps.append({
            "x": asc(x[b].reshape(C, N).astype(np.float32)),
            "wqt4": wqt4, "wkt4": wkt4, "wvt": wvt,
            "bq4": bq4, "bk4": bk4, "bv2": bv2, "gamma128": g128,
        })
    return maps


def _run(inputs, trace=False):
    nc = _build()
    in_maps = _prep_in_maps(**{k: np.asarray(v) for k, v in inputs.items()})
    res = run_bass_kernel_spmd(nc, in_maps, list(range(NCORES)), trace=trace)
    out = np.stack([res.results[b]["out"].reshape(C, H, W) for b in range(B)])
    return out.astype(np.float32), res


def kernel(**inputs):
    out, _ = _run(inputs, trace=False)
    return out


# revision 44
# speedup vs baseline: 1.1158x; 1.1158x over previous
"""Multi-head self-attention (1x1-conv projections, N=4096 spatial tokens,
C=256 channels, Cq=32) on 8 TRN2 NeuronCores, data-parallel over batch.

Per core (one batch element, x as [C, N]):
  q = wq @ x + bq          [Cq, N]
  k = wk @ x + bk          [Cq, N]
  v = wv @ x               [C, N]   (bv folded into the epilogue)
  S = q^T k                [N, N]
  P = softmax(S, axis=-1)
  out = gamma * (v @ P^T + bv) + x

Layout strategy: compute S^T tiles (keys j on partitions, queries i on the
free dim) so softmax's exp output E^T feeds the PV matmul as the stationary
operand with rhs = [v^T | ones]; the ones column accumulates the softmax
denominator for free (no P transposes, no separate reduction). exp skips
max-subtraction: S ~ N(0, 32), |S| < ~40 stays far inside fp32 exp range.

dtypes: fp32r (tf32-like, full PE speed at moving-dim>=256) for the
q/k/energy path where exp amplifies absolute error; bf16 for the P*V path
where softmax normalization cancels it.
"""

import numpy as np

import concourse.bass as bass
import concourse.mybir as mybir
import concourse.tile as tile
from concourse.bass_utils import run_bass_kernel_spmd
from concourse.masks import make_identity
from concourse.tile import ScopedClock

F32 = mybir.dt.float32
F32R = mybir.dt.float32r
BF16 = mybir.dt.bfloat16

B, C, CQ = 8, 256, 32
H = W = 64
N = H * W            # 4096 tokens
NCORES = 8
CT = C // 128        # 2 channel tiles
IB = 512             # queries per i-block
N_IB = N // IB       # 8
JT = N // 128        # 32 key tiles
JGRP = 4             # key tiles per exp group (one PSUM S tile = 4 banks)
N_JG = JT // JGRP    # 8


class PatchedTileContext(tile.TileContext):
    """This walrus build supports only ONE sync-wait command per
    instruction. Peel extra waits into standalone single-wait NOPs on the
    same engine queue, emitted immediately before the instruction (a serial
    conjunction of waits - semantically identical). Same treatment for the
    kernel-tail drain, whose global-clock waits otherwise all land on one
    Drain instruction."""

    MAX_WAITS_PER_INST = 1

    def _add_instruction(self, inst):
        si = inst.sync_info
        waits = list(si.on_wait) if si is not None and si.on_wait else []
        if len(waits) > self.MAX_WAITS_PER_INST and inst.engine is not None:
            keep = waits[-self.MAX_WAITS_PER_INST:]
            peel = waits[: -self.MAX_WAITS_PER_INST]
            for w in peel:
                nop = mybir.InstNoOp(
                    name=self.nc.get_next_instruction_name(),
                    ins=[],
                    outs=[],
                    sync_info=mybir.SyncInfo(on_wait=[w], on_update=[]),
                )
                nop.engine = inst.engine
                super()._add_instruction(nop)
            inst.sync_info = mybir.SyncInfo(
                on_wait=keep,
                on_update=list(si.on_update) if si.on_update else [],
            )
        super()._add_instruction(inst)

    def _drain_and_barrier(self, tick_clock, wait_clock):
        nc = self.nc
        carrier = nc.sync.nop()
        wait_clock.add_sem_waits(
            carrier.ins, ScopedClock({None: tick_clock.global_clock})
        )
        si = carrier.ins.sync_info
        waits = list(si.on_wait) if si is not None and si.on_wait else []
        carrier.ins.sync_info = None
        for w in waits:
            h = bass.SemaphoreHandle(name=w.ant_name or f"sem{w.id}", num=w.id)
            if w.wait_mode == "sem-ge-imm":
                nc.sync.wait_ge(h, w.wait_value)
            else:
                op = {
                    "sem-eq-imm": "eq",
                    "sem-le-imm": "le",
                    "sem-lt-imm": "lt",
                    "sem-gt-imm": "gt",
                }[w.wait_mode]
                nc.sync.wait_op(h, w.wait_value, op)
        nc.sync.drain()
        nc.all_engine_barrier()
        assert self.sems is not None
        popped = nc._tile_sem_poison_stack.pop()
        assert popped is self._sem_poison
        nc.clear_and_free_semaphores(list(self.sems.allocated().values()))
        nc.all_engine_barrier()


def _attention_body(nc, tc, ctx):
    x_e = nc.dram_tensor("x", [C, N], F32, kind="ExternalInput")
    wqt4_e = nc.dram_tensor("wqt4", [C, 128], F32, kind="ExternalInput")
    wkt4_e = nc.dram_tensor("wkt4", [C, 128], F32, kind="ExternalInput")
    wvt_e = nc.dram_tensor("wvt", [C, C], F32, kind="ExternalInput")
    bq4_e = nc.dram_tensor("bq4", [128, 1], F32, kind="ExternalInput")
    bk4_e = nc.dram_tensor("bk4", [128, 1], F32, kind="ExternalInput")
    bv_e = nc.dram_tensor("bv2", [128, CT], F32, kind="ExternalInput")
    gamma_e = nc.dram_tensor("gamma128", [128, 1], F32, kind="ExternalInput")
    out_e = nc.dram_tensor("out", [C, N], F32, kind="ExternalOutput")

    x_v = x_e.rearrange("(t p) n -> p t n", p=128)      # [128, CT, N]
    out_v = out_e.rearrange("(t p) n -> p t n", p=128)  # [128, CT, N]
    wqt_v = wqt4_e.rearrange("(t p) m -> p t m", p=128)
    wkt_v = wkt4_e.rearrange("(t p) m -> p t m", p=128)
    wvt_v = wvt_e.rearrange("(t p) m -> p t m", p=128)

    const = ctx.enter_context(tc.tile_pool(name="const", bufs=1))
    sb = ctx.enter_context(tc.tile_pool(name="sb", bufs=1))
    eps = ctx.enter_context(tc.tile_pool(name="eps", bufs=4))
    outp = ctx.enter_context(tc.tile_pool(name="outp", bufs=4))

    # ---- constants / weights ----
    bq4 = const.tile([128, 1], F32)
    bk4 = const.tile([128, 1], F32)
    bv2 = const.tile([128, CT], F32)
    gamma = const.tile([128, 1], F32)
    nc.gpsimd.dma_start(out=bq4, in_=bq4_e[:, :])
    nc.gpsimd.dma_start(out=bk4, in_=bk4_e[:, :])
    nc.gpsimd.dma_start(out=bv2, in_=bv_e[:, :])
    nc.gpsimd.dma_start(out=gamma, in_=gamma_e[:, :])

    wq_f = const.tile([128, CT, 128], F32)
    wk_f = const.tile([128, CT, 128], F32)
    wv_f = const.tile([128, CT, C], F32)
    nc.scalar.dma_start(out=wq_f, in_=wqt_v)
    nc.scalar.dma_start(out=wk_f, in_=wkt_v)
    nc.scalar.dma_start(out=wv_f, in_=wvt_v)
    wq_r = const.tile([128, CT, 128], F32R)
    wk_r = const.tile([128, CT, 128], F32R)
    wv_r = const.tile([128, CT, C], F32R)
    nc.vector.tensor_copy(out=wq_r, in_=wq_f)
    nc.vector.tensor_copy(out=wk_r, in_=wk_f)
    nc.vector.tensor_copy(out=wv_r, in_=wv_f)

    ident = const.tile([128, 128], BF16)
    make_identity(nc, ident)

    # ---- x load + fp32r round + projections, pipelined in 512-col chunks ----
    x_sb = sb.tile([128, CT, N], F32)
    xf_r = sb.tile([128, CT, N], F32R)
    qT = sb.tile([128, N], F32R)   # q^T replicated on 4 partition groups
    kT = sb.tile([128, N], F32R)
    v1T = sb.tile([128, JT, C + 1], BF16)  # [j-part, j-tile, c | ones]

    # ACT exp-table preload: dummy exp (output overwritten by the x_sb
    # load, which gives the location a reader) pulls the ~2.7us table DMA
    # into the input-load window instead of stalling the first softmax
    zt = const.tile([128, 1], F32)
    nc.vector.memset(zt, 0.0)
    nc.scalar.activation(
        out=x_sb[:, 0, 0:1], in_=zt, func=mybir.ActivationFunctionType.Exp
    )

    with tc.tile_pool(name="psA", bufs=6, space="PSUM") as psA:
        # HAM warm-up: ~4us of dependency-free back-to-back matmuls so the
        # PE clock gate opens (1.2 -> 2.4 GHz) before the real work lands
        wu = const.tile([128, 1024], BF16)
        nc.vector.memset(wu, 0.0)
        pwu = psA.tile([128, 512], F32, tag="pj", name="pwu")
        for _ in range(9):
            nc.tensor.matmul(
                pwu, wu[:, 0:128], wu[:, 0:512], start=True, stop=True
            )
        for ch in range(8):
            sl = bass.ts(ch, 512)
            nc.sync.dma_start(out=xf_r[:, :, sl], in_=x_v[:, :, sl].bitcast(F32R))
            pq = psA.tile([128, 512], F32, tag="pj")
            nc.tensor.matmul(pq, wq_r[:, 0, :], xf_r[:, 0, sl], start=True, stop=False)
            nc.tensor.matmul(pq, wq_r[:, 1, :], xf_r[:, 1, sl], start=False, stop=True)
            nc.vector.tensor_scalar(
                out=qT[:, sl], in0=pq, scalar1=bq4, scalar2=None,
                op0=mybir.AluOpType.add,
            )
            pk = psA.tile([128, 512], F32, tag="pj")
            nc.tensor.matmul(pk, wk_r[:, 0, :], xf_r[:, 0, sl], start=True, stop=False)
            nc.tensor.matmul(pk, wk_r[:, 1, :], xf_r[:, 1, sl], start=False, stop=True)
            nc.vector.tensor_scalar(
                out=kT[:, sl], in0=pk, scalar1=bk4, scalar2=None,
                op0=mybir.AluOpType.add,
            )
            for nt in range(ch * 4, ch * 4 + 4):
                pv = psA.tile([128, C], F32, tag="pj")
                nc.tensor.matmul(
                    pv, xf_r[:, 0, bass.ts(nt, 128)], wv_r[:, 0, :],
                    start=True, stop=False,
                )
                nc.tensor.matmul(
                    pv, xf_r[:, 1, bass.ts(nt, 128)], wv_r[:, 1, :],
                    start=False, stop=True,
                )
                nc.scalar.copy(out=v1T[:, nt, 0:C], in_=pv)
        nc.vector.memset(v1T[:, :, C : C + 1], 1.0)

    # residual load: off the critical path, overlaps early attention work
    nc.sync.dma_start(out=x_sb, in_=x_v)

    # xb = x + gamma*bv  (residual with bv folded in; written in place)
    gbv = const.tile([128, CT], F32)
    nc.vector.tensor_scalar(
        out=gbv, in0=bv2, scalar1=gamma, scalar2=None, op0=mybir.AluOpType.mult
    )
    for t in range(CT):
        nc.vector.tensor_scalar(
            out=x_sb[:, t, :], in0=x_sb[:, t, :], scalar1=gbv[:, t : t + 1],
            scalar2=None, op0=mybir.AluOpType.add,
        )

    # ---- attention ----
    E = sb.tile([128, JT, IB], BF16)  # exp(S^T) for one i-block

    def emit_energy(ib, jg):
        # S^T for 4 key-tiles (row-packed K=32 matmuls); exp in 2 halves so
        # PV can start on the first pair of key-tiles while the second is
        # still in the ACT pipe
        isl = bass.ds(ib * IB, IB)
        halves = [
            psS.tile([128, JGRP // 2, IB], F32, tag=f"S{h}", name=f"S_{ib}_{jg}_{h}")
            for h in range(2)
        ]
        for g in range(JGRP):  # back-to-back, one bank each: 4-way row-group
            jt = jg * JGRP + g  # concurrency in the PE array
            gp = bass.ds(32 * g, 32)
            nc.tensor.matmul(
                halves[g // 2][:, g % 2, :],
                kT[gp, bass.ts(jt, 128)],
                qT[gp, isl],
                start=True, stop=True,
                tile_position=(32 * g, 0),
            )
        for h in range(2):
            nc.scalar.activation(
                out=E[:, jg * JGRP + h * 2 : jg * JGRP + h * 2 + 2, :],
                in_=halves[h][:, :, :],
                func=mybir.ActivationFunctionType.Exp,
            )

    with (
        tc.tile_pool(name="psS", bufs=1, space="PSUM") as psS,
        tc.tile_pool(name="psO", bufs=4, space="PSUM") as psO,
    ):
        emit_energy(0, 0)
        for ib in range(N_IB):
            po = [
                psO.tile([128, C + 1], F32, tag="acc", name=f"po_{ib}_{i_s}")
                for i_s in range(4)
            ]
            for jg in range(N_JG):
                # software pipeline: queue the NEXT group's energy+exp ahead
                # of this group's PV matmuls so ACT overlaps the PE stream
                if jg + 1 < N_JG:
                    emit_energy(ib, jg + 1)
                elif ib + 1 < N_IB:
                    emit_energy(ib + 1, 0)
                for g in range(JGRP):
                    jt = jg * JGRP + g
                    for i_s in range(4):
                        nc.tensor.matmul(
                            po[i_s],
                            E[:, jt, bass.ts(i_s, 128)],
                            v1T[:, jt, :],
                            start=(jt == 0), stop=(jt == JT - 1),
                        )
            # epilogue: normalize, transpose to [c, n], residual, store
            for i_s in range(4):
                rd = eps.tile([128, 1], F32, tag="rd")
                nc.vector.reciprocal(out=rd, in_=po[i_s][:, C : C + 1])
                nc.vector.tensor_mul(out=rd, in0=rd, in1=gamma)
                pvn = eps.tile([128, C], BF16, tag="pvn")
                nc.vector.tensor_scalar(
                    out=pvn, in0=po[i_s][:, 0:C], scalar1=rd, scalar2=None,
                    op0=mybir.AluOpType.mult,
                )
                pt = psO.tile([128, C], BF16, tag="acc")
                nc.tensor.transpose(pt[:, 0:128], pvn[:, 0:128], ident)
                nc.tensor.transpose(pt[:, 128:256], pvn[:, 128:256], ident)
                for t in range(CT):
                    ot = outp.tile([128, 128], F32, tag="ot")
                    nc.vector.tensor_add(
                        out=ot,
                        in0=pt[:, bass.ts(t, 128)],
                        in1=x_sb[:, t, bass.ds(ib * IB + i_s * 128, 128)],
                    )
                    nc.sync.dma_start(
                        out=out_v[:, t, bass.ds(ib * IB + i_s * 128, 128)], in_=ot
                    )


_CACHE = {}


def _build():
    if "nc" not in _CACHE:
        nc = bass.Bass()
        from contextlib import ExitStack
        with PatchedTileContext(nc) as tc, ExitStack() as ctx:
            _attention_body(nc, tc, ctx)
        _CACHE["nc"] = nc
    return _CACHE["nc"]


def _prep_in_maps(x, wq, bq, wk, bk, wv, bv, gamma):
    asc = np.ascontiguousarray
    wqt4 = asc(np.tile(wq, (4, 1)).T.astype(np.float32))    # [C, 128]
    wkt4 = asc(np.tile(wk, (4, 1)).T.astype(np.float32))    # [C, 128]
    wvt = asc(wv.T.astype(np.float32))                      # [C, C]
    bq4 = asc(np.tile(bq, 4)[:, None].astype(np.float32))   # [128, 1]
    bk4 = asc(np.tile(bk, 4)[:, None].astype(np.float32))
    bv2 = asc(bv.reshape(CT, 128).T.astype(np.float32))     # [128, CT]
    g128 = np.full((128, 1), np.float32(gamma[0]), dtype=np.float32)
    maps = []
    for b in range(B):
        maps.append({
            "x": asc(x[b].reshape(C, N).astype(np.float32)),
            "wqt4": wqt4, "wkt4": wkt4, "wvt": wvt,
            "bq4": bq4, "bk4": bk4, "bv2": bv2, "gamma128": g128,
        })
    return maps


def _run(inputs, trace=False):
    nc = _build()
    in_maps = _prep_in_maps(**{k: np.asarray(v) for k, v in inputs.items()})
    res = run_bass_kernel_spmd(nc, in_maps, list(range(NCORES)), trace=trace)
    out = np.stack([res.results[b]["out"].reshape(C, H, W) for b in range(B)])
    return out.astype(np.float32), res


def kernel(**inputs):
    out, _ = _run(inputs, trace=False)
    return out
